# revision 47
# baseline (speedup 1.0000x reference)
"""Trainium2 Bass kernel for nn_AdaptiveSampler (sparse grid_sample attention).

Strategy v4.7 (data-parallel over batch, 8 cores x 4 batch items each):
  - Host: features channels-last bf16 in DRAM; all gather indices derive
    from keypoint_coords only and are precomputed on host.
  - Seed path: bf16 channel-major transposed gathers (2-cell rows x 2 y
    rows), DVE bilinear combine in 2x mode, y-add writes the seed as fp8
    in (e', s, i) pair layout, then the C->128 MLP layer runs as fp8
    DoubleRow matmuls (2x column rate).
  - Patch path y-adaptive: keypoints whose frac(iy) is within DELTA of an
    integer boundary ("B class") need a 3-row window; the rest ("A") need
    only 2 rows (saves ~23% of patch bytes). Per batch item the keypoints
    are permuted B-first; one gather per b fetches [row0 x128 | row1 x128 |
    row2 x NBCAP]. The permutation back to original keypoint order is
    folded into the fuse stationaries (permutation matrix instead of
    identity), so it costs nothing.
  - Fuse weights use the exact tent form relu(1 - |p - cell|) (no
    floor/is_equal chains); softmax runs without the max-subtraction
    (logits are tiny). DVE emission order is tuned so weights_pair(0),
    which gates the whole fuse chain, runs before the half-1 combine tail.
  - Fuse: 9 permuted-diag matmuls per batch item on PE (bf16); the 3
    third-row cells contract only over the first NBCAP=64 partitions.
  - Gathers: all 4 seed calls first, then the 4 patch calls — the seed-fed
    MLP/weights pipeline then overlaps the patch drains, and every fuse is
    gated only by its own patch arrival. Gather calls are merged (8 total)
    because each SWDGE call carries ~1-3.5us of serial overhead; transposed
    gathers above 256 idxs hang the hardware, so seeds stay at 4 calls.
"""

import os
import sys
from contextlib import ExitStack

import numpy as np

sys.path.insert(0, "/opt/trn_rl_repo")

import ml_dtypes

import concourse.bass as bass
import concourse.tile as tile
from concourse import bacc, mybir

F32 = mybir.dt.float32
BF16 = mybir.dt.bfloat16
F8 = mybir.dt.float8e4
I16 = mybir.dt.int16

ALU = mybir.AluOpType
ACT = mybir.ActivationFunctionType
AX = mybir.AxisListType
DR = mybir.MatmulPerfMode.DoubleRow

B = 4          # batch items per core
C = 1024       # channels
H = W = 64
HW = H * W     # 4096 cells per batch item
J = 128        # keypoints
NP = 4         # sample points per keypoint
TWO23 = float(2 ** 23)

DELTA = 0.12   # y-window classification margin (max |offset| is ~0.080)
NBCAP = 64     # static cap on 3-row keypoints per batch item (max seen: 39);
               # 64 keeps the row-2 fuse matmuls quarter-aligned (K=64) and
               # every row-2 partition gather-initialized (no memset needed)

# bpack bf16 column map
BP_ID8 = 0        # eye(8) rows 0:8
BP_W2O = 8        # [8]
BP_W2A = 16       # [4]
BP_SIDX = 20      # seed idx 64 cols
BP_PIDX = 84      # patch idx 4 * (16 + NBCAP//16) cols
BP_TOT = 84 + 4 * (16 + NBCAP // 16)


def build_nc():
    nc = bacc.Bacc()

    feat = nc.declare_dram_parameter("feat", [B * HW, C], BF16, isOutput=False)
    # seed-combine weights wsg[x, y, jb] replicated across partitions
    wsg = nc.declare_dram_parameter("wsg", [128, 2 * 2 * J * B], BF16,
                                    isOutput=False)
    # fp8 W1 pack: [head(2), e(4), s(2), m(128)] per partition row
    w1pk_d = nc.declare_dram_parameter("w1pk", [128, 2048], F8, isOutput=False)
    # per-b permutation matrices (sigma-row -> original keypoint)
    ppack = nc.declare_dram_parameter("ppack", [128, 4 * 128], BF16,
                                      isOutput=False)
    # bf16 pack: [id8 | w2o 8 | w2a 4 | seed idx 64 | patch idx]
    bpack = nc.declare_dram_parameter("bpack", [128, BP_TOT], BF16,
                                      isOutput=False)
    # f32 pack: [ix 4 | iy 4 | bx 4 | ylo 4 | b1o | b1a | posc 4 | b2o | b2a]
    fpack = nc.declare_dram_parameter("fpack", [128, 24], F32, isOutput=False)
    out = nc.declare_dram_parameter("out", [B * J, C], BF16, isOutput=True)

    # Overlapping row views. 3-cell rows (patch): max start idx 16381.
    # 2-cell rows (seed): max start 16382.
    feat_ov3 = bass.AP(feat[:].tensor, 0, [[C, B * HW - 2], [1, 3 * C]])
    feat_ov2 = bass.AP(feat[:].tensor, 0, [[C, B * HW - 1], [1, 2 * C]])

    with ExitStack() as ctx:
        tc = ctx.enter_context(tile.TileContext(nc))
        cons = ctx.enter_context(tc.tile_pool(name="cons", bufs=1))
        gp = ctx.enter_context(tc.tile_pool(name="gpool", bufs=1))
        a = ctx.enter_context(tc.tile_pool(name="work", bufs=1))
        dgp = ctx.enter_context(tc.tile_pool(name="diag", bufs=2))
        ps = ctx.enter_context(tc.tile_pool(name="psT", bufs=2, space="PSUM"))
        pmm = ctx.enter_context(tc.tile_pool(name="psMM", bufs=2, space="PSUM"))
        pfu = ctx.enter_context(tc.tile_pool(name="psFU", bufs=2, space="PSUM"))

        # ---------------- constants ----------------
        # The idx-bearing pack goes first so the gathers unblock ASAP.
        bpk = cons.tile([128, BP_TOT], BF16, tag="bpk")
        nc.scalar.dma_start(out=bpk[:], in_=bpack[:])
        sidx_sb = bpk[:, BP_SIDX:BP_SIDX + 64].bitcast(I16)
        pidx_sb = bpk[:, BP_PIDX:BP_TOT].bitcast(I16)
        PIW = 16 + NBCAP // 16  # idx cols per batch item

        # ---------------- gathers ----------------------------------------
        # Seed chunks (bf16, channel-major, transposed): h -> y = h//2,
        # jb half = h%2, 256 idxs each.
        # Patch (keypoint-major, y-adaptive): one call per b of
        # [row0 x128 | row1 x128 | row2 x NBCAP]; rows 0..NBCAP-1 of the
        # third stripe are always gather-initialized and the row-2 fuse
        # matmuls contract K=NBCAP only.
        G2h = [None] * 4
        Gt = [None] * B

        def seed_gather(h):
            # bf16 so the DVE combine runs in 2x mode; channel = 128*e16 + p
            # with e16 = x*8 + q.
            g2 = gp.tile([128, 16, 256], BF16, tag=f"G2{h}")
            nc.gpsimd.dma_gather(
                g2[:],
                feat_ov2,
                sidx_sb[:, 16 * h:16 * h + 16],
                num_idxs=256,
                num_idxs_reg=r256,
                elem_size=2 * C,
                elem_step=C,
                transpose=True,
            )
            G2h[h] = g2

        def patch_gather(b):
            g = gp.tile([128, 3, 3 * C], BF16, tag=f"G{b}")
            nc.gpsimd.dma_gather(
                g[:],
                feat_ov3,
                pidx_sb[:, b * PIW:(b + 1) * PIW],
                num_idxs=256 + NBCAP,
                num_idxs_reg=r320,
                elem_size=3 * C,
                elem_step=C,
                transpose=False,
            )
            Gt[b] = g

        r256 = nc.gpsimd.to_reg(256)
        r320 = nc.gpsimd.to_reg(256 + NBCAP)

        seed_gather(0)
        seed_gather(2)
        seed_gather(1)
        seed_gather(3)
        patch_gather(0)
        patch_gather(1)
        patch_gather(2)
        patch_gather(3)

        # ---------------- remaining constants (overlap the gathers) -------
        fpk = cons.tile([128, 24], F32, tag="fpk")
        nc.sync.dma_start(out=fpk[:], in_=fpack[:])
        wsg_flat = cons.tile([128, 2 * 2 * J * B], BF16, tag="wsg")
        nc.sync.dma_start(out=wsg_flat[:], in_=wsg[:])
        wsg_sb = wsg_flat[:].rearrange("p (x y i) -> p x y i", x=2, y=2)
        w1pk = cons.tile([128, 2048], F8, tag="w1pk")
        nc.sync.dma_start(out=w1pk[:], in_=w1pk_d[:])
        w1v = w1pk[:].rearrange("p (hd e s m) -> p hd e s m", hd=2, e=4, s=2,
                                m=128)
        ppk = cons.tile([128, 512], BF16, tag="ppk")
        nc.sync.dma_start(out=ppk[:], in_=ppack[:])

        id8_sb = bpk[0:8, 0:8]
        w2o_sb = bpk[:, BP_W2O:BP_W2O + 8]
        w2a_sb = bpk[:, BP_W2A:BP_W2A + 4]
        b1o_sb = fpk[:, 16:17]
        b1a_sb = fpk[:, 17:18]
        posc_sb = fpk[:, 18:22]
        b2o_sb = fpk[0:8, 22:23]
        b2a_sb = fpk[0:4, 23:24]

        ixv = fpk[:, 0:4]    # [J, B] pixel x coords (sigma order)
        iyv = fpk[:, 4:8]
        bxv = fpk[:, 8:12]   # patch x base (f32 integer-valued)
        ylov = fpk[:, 12:16]  # patch y base (y0 for A class, by for B class)

        # ---------------- seed combine (DVE, fp8) ------------------------
        # G2 layout: [p, x(2), e(4), i(256), s(2)], channel = 2*(128e+p)+s
        seedh = []
        for jh in range(2):
            sh = a.tile([128, 4, 256, 2], F8, tag=f"seed{jh}")
            seedh.append(sh)

        comb_t = {}

        def combine_y(jh, y):
            g2 = G2h[2 * y + jh]
            vg = (g2[:].rearrange("p a i -> p (a i)")
                  .rearrange("p (x q i) -> p x q i", x=2, q=8, i=256))
            t = a.tile([128, 8, 256], BF16, tag=f"tcomb{jh}_{y}")
            t2 = a.tile([128, 8, 256], BF16, tag=f"tcomb2{jh}_{y}")
            for x in range(2):
                wb = (wsg_sb[:, x, y, 256 * jh:256 * jh + 256]
                      .unsqueeze(1).to_broadcast((128, 8, 256)))
                nc.vector.tensor_tensor(
                    (t if x == 0 else t2)[:], vg[:, x], wb, ALU.mult)
            nc.vector.tensor_tensor(t[:], t[:], t2[:], ALU.add)
            comb_t[(jh, y)] = t

        def combine_fin(jh):
            # y-add writes the fp8 seed in (e', s, i) pair layout for the
            # DoubleRow MLP: channel = 128*(2e'+s) + p, q = 2e'+s.
            sv = (seedh[jh][:].rearrange("p e i s -> p (e i s)")
                  .rearrange("p (e i s) -> p e s i", e=4, i=256, s=2))
            t0v = comb_t[(jh, 0)][:].rearrange("p (e s) i -> p e s i",
                                               e=4, s=2)
            t1v = comb_t[(jh, 1)][:].rearrange("p (e s) i -> p e s i",
                                               e=4, s=2)
            nc.vector.tensor_tensor(sv, t0v, t1v, ALU.add)

        def combine_half(jh):
            combine_y(jh, 0)
            combine_y(jh, 1)
            combine_fin(jh)

        # DVE emission order matters (in-order engine): combine half-0 fully,
        # then only the y0-part of half-1, so weights_pair(0) — which gates
        # the whole fuse chain — runs as soon as the half-0 MLP lands.
        with nc.allow_low_precision("fp8 grid-sample compute"):
            combine_half(0)
            combine_y(1, 0)

        # ---------------- MLP + w9 weights, per jb-half pipeline ----------
        offT = a.tile([J, B, 8], F32)
        attT = a.tile([J, B, 4], F32)

        def mlp_chain(jh):
            seedv = seedh[jh][:].rearrange("p e i s -> p e s i")

            def head(hd, b1_sb, w2_sb, b2_sb, m2, name):
                hps = pmm.tile([128, 256], F32, tag="mlp")
                for e in range(4):
                    nc.tensor.matmul(
                        hps[:], w1v[:, hd, e], seedv[:, e],
                        start=(e == 0), stop=(e == 3), perf_mode=DR,
                    )
                h_sb = a.tile([128, 256], BF16, tag=f"hsb_{name}{jh}")
                nc.scalar.activation(h_sb[:], hps[:], ACT.Relu, bias=b1_sb)
                ps2 = pmm.tile([m2, 256], F32, tag="mlp")
                nc.tensor.matmul(ps2[:], w2_sb, h_sb[:], start=True, stop=True)
                o2 = a.tile([m2, 256], BF16, tag=f"o2_{name}{jh}")
                nc.scalar.activation(o2[:], ps2[:], ACT.Identity, bias=b2_sb)
                return o2

            off2 = head(0, b1o_sb, w2o_sb, b2o_sb, 8, "off")
            att2 = head(1, b1a_sb, w2a_sb, b2a_sb, 4, "att")
            for bl in range(2):
                b = 2 * jh + bl
                pso = ps.tile([128, 8], BF16, tag="tp")
                nc.tensor.transpose(
                    pso[:, 0:8], off2[:, bl * J:(bl + 1) * J], id8_sb)
                nc.scalar.copy(offT[:, b, :], pso[:, 0:8])
                psa = ps.tile([128, 4], BF16, tag="tp")
                nc.tensor.transpose(
                    psa[:, 0:4], att2[:, bl * J:(bl + 1) * J],
                    id8_sb[0:4, 0:4])
                nc.scalar.copy(attT[:, b, :], psa[:, 0:4])

        def weights_pair(jh):
            """Fuse weights for batch pair (2jh, 2jh+1) -> w9 f32 [J,2,3,3].

            Axis weights use the exact tent form relu(1 - |p - cell|), which
            equals the reference's position-select bilinear weights including
            the out-of-window and clipped-border zero cases.
            """
            bsl = slice(2 * jh, 2 * jh + 2)
            # pxy[j, b, n, ax]: sample positions for both axes
            pxy = a.tile([J, 2, NP, 2], F32, tag=f"pxy{jh}")
            nc.vector.tensor_tensor(
                pxy[:, :, :, 0],
                ixv[:, bsl].unsqueeze(2).to_broadcast((J, 2, NP)),
                offT[:, bsl, 0:NP],
                ALU.add,
            )
            nc.vector.tensor_tensor(
                pxy[:, :, :, 1],
                iyv[:, bsl].unsqueeze(2).to_broadcast((J, 2, NP)),
                offT[:, bsl, NP:2 * NP],
                ALU.add,
            )
            # basepos[j, b, ax, pos] = window base + pos
            posb3 = posc_sb[:, 0:3].unsqueeze(1).to_broadcast((J, 2, 3))
            basepos = a.tile([J, 2, 2, 3], F32, tag=f"bp{jh}")
            nc.vector.tensor_tensor(
                basepos[:, :, 0, :],
                bxv[:, bsl].unsqueeze(2).to_broadcast((J, 2, 3)), posb3,
                ALU.add,
            )
            nc.vector.tensor_tensor(
                basepos[:, :, 1, :],
                ylov[:, bsl].unsqueeze(2).to_broadcast((J, 2, 3)), posb3,
                ALU.add,
            )
            # tent: sel = relu(1 - |pxy - basepos|)  [J, 2, NP, 2, 3]
            sel = a.tile([J, 2, NP, 2, 3], F32, tag=f"sel{jh}")
            nc.vector.tensor_tensor(
                sel[:],
                pxy[:].unsqueeze(4).to_broadcast((J, 2, NP, 2, 3)),
                basepos[:].unsqueeze(2).to_broadcast((J, 2, NP, 2, 3)),
                ALU.subtract,
            )
            neg = a.tile([J, 2, NP, 2, 3], F32, tag=f"neg{jh}")
            nc.vector.tensor_scalar(neg[:], sel[:], -1.0, 0.0, ALU.mult,
                                    ALU.add)
            nc.vector.tensor_tensor(sel[:], sel[:], neg[:], ALU.max)
            nc.vector.tensor_scalar(sel[:], sel[:], -1.0, 1.0, ALU.mult,
                                    ALU.add)
            nc.vector.tensor_scalar_max(sel[:], sel[:], 0.0)
            wxsel = sel[:, :, :, 0, :]
            wysel = sel[:, :, :, 1, :]
            amax = a.tile([J, 2, 1], F32, tag=f"amax{jh}")
            nc.vector.tensor_reduce(amax[:], attT[:, bsl, :], AX.X, ALU.max)
            ae = a.tile([J, 2, NP], F32, tag=f"ae{jh}")
            nc.vector.tensor_tensor(
                ae[:], attT[:, bsl, :], amax[:].to_broadcast((J, 2, NP)),
                ALU.subtract,
            )
            nc.scalar.activation(ae[:], ae[:], ACT.Exp)
            asum = a.tile([J, 2, 1], F32, tag=f"asum{jh}")
            nc.vector.tensor_reduce(asum[:], ae[:], AX.X, ALU.add)
            nc.vector.reciprocal(asum[:], asum[:])
            attw = a.tile([J, 2, NP], F32, tag=f"attw{jh}")
            nc.vector.tensor_tensor(
                attw[:], ae[:], asum[:].to_broadcast((J, 2, NP)), ALU.mult
            )
            ty = a.tile([J, 2, NP, 3], F32, tag=f"ty{jh}")
            nc.vector.tensor_tensor(
                ty[:], wysel,
                attw[:].unsqueeze(3).to_broadcast((J, 2, NP, 3)),
                ALU.mult,
            )
            w9 = a.tile([J, 2, 3, 3], F32, tag=f"w9{jh}")
            tmp9 = a.tile([J, 2, 3, 3], F32, tag=f"tmp9{jh}")
            for n in range(NP):
                dst = (w9 if n == 0 else tmp9)
                nc.vector.tensor_tensor(
                    dst[:],
                    ty[:, :, n, :].unsqueeze(3).to_broadcast((J, 2, 3, 3)),
                    wxsel[:, :, n, :].unsqueeze(2).to_broadcast((J, 2, 3, 3)),
                    ALU.mult,
                )  # wxsel slice dims [J, 2, 3] -> bcast over y
                if n > 0:
                    nc.vector.tensor_tensor(w9[:], w9[:], tmp9[:], ALU.add)
            return w9

        # ---------------- fuse: permuted-diag matmuls ---------------------
        def build_dgs(b, w9, bl, w9b=None):
            # Pairs 0-1 build stationaries on Scalar; pairs 2-3 on GpSimd
            # (idle once the gather DGE work drains), so the late fuses
            # aren't queued behind the Scalar fo-copies.
            p_sb = ppk[:, 128 * b:128 * (b + 1)]
            dgs = []
            for k in range(9):
                dg = dgp.tile([128, 128], BF16, tag=f"dg{k}",
                              padded_shape=[128, 1024])
                if w9b is None:
                    nc.scalar.activation(
                        dg[:], p_sb, ACT.Identity,
                        scale=w9[:, bl, k // 3, k % 3:k % 3 + 1],
                    )
                else:
                    nc.gpsimd.tensor_tensor(
                        dg[:], p_sb,
                        w9b[:, bl, k:k + 1].to_broadcast((128, 128)),
                        ALU.mult,
                    )
                dgs.append(dg)
            return dgs

        def fuse_b(b, dgs):
            # acc0's nine cells complete before acc1 starts, so the first
            # half of the output evacuates while acc1 still accumulates.
            acc0 = pfu.tile([128, 512], F32, tag="facc0")
            acc1 = pfu.tile([128, 512], F32, tag="facc1")
            for acc, hh in ((acc0, 0), (acc1, 1)):
                for k in range(9):
                    y, x = k // 3, k % 3
                    kext = 128 if y < 2 else NBCAP
                    nc.tensor.matmul(
                        acc[:], dgs[k][0:kext, :],
                        Gt[b][0:kext, y,
                              x * C + hh * 512:x * C + hh * 512 + 512],
                        start=(k == 0), stop=(k == 8),
                        skip_group_check=True,
                    )
            return acc0, acc1

        def out_b(b, acc0, acc1):
            fo = a.tile([128, C], BF16, tag=f"fo{b}")
            nc.scalar.copy(fo[:, 0:512], acc0[:])
            nc.scalar.copy(fo[:, 512:1024], acc1[:])
            nc.sync.dma_start(out=out[b * J:(b + 1) * J, :], in_=fo[:])

        mlp_chain(0)
        w90 = weights_pair(0)
        with nc.allow_low_precision("fp8 grid-sample compute"):
            combine_y(1, 1)
            combine_fin(1)
        mlp_chain(1)
        w91 = weights_pair(1)
        w9b1 = a.tile([J, 2, 9], BF16, tag="w9b1")
        nc.vector.tensor_copy(
            w9b1[:], w91[:].rearrange("j b y x -> j b (y x)"))
        dgs0 = build_dgs(0, w90, 0)
        a0, a1 = fuse_b(0, dgs0)
        dgs1 = build_dgs(1, w90, 1)
        b0, b1 = fuse_b(1, dgs1)
        out_b(0, a0, a1)
        dgs2 = build_dgs(2, w91, 0, w9b=w9b1)
        c0, c1 = fuse_b(2, dgs2)
        out_b(1, b0, b1)
        dgs3 = build_dgs(3, w91, 1, w9b=w9b1)
        d0, d1 = fuse_b(3, dgs3)
        out_b(2, c0, c1)
        out_b(3, d0, d1)

    nc.finalize()
    return nc


def prepare_in_maps(features, keypoint_coords, w_off1, b_off1, w_off2, b_off2,
                    w_att1, b_att1, w_att2, b_att2, n_cores=8):
    bf = ml_dtypes.bfloat16
    f8 = ml_dtypes.float8_e4m3
    f32 = np.float32

    def wrap(flat):  # [N] int16 -> [128, N//16] gpsimd wrapped layout
        n = flat.shape[0]
        return np.tile(flat.reshape(n // 16, 16).T, (8, 1))

    # W1 pack: [p, head, e, s, m] with channel(e, p, s) = 128*(2e+s)+p
    def w1f8(w):  # [128m, C] -> [128p, 4e, 2s, 128m]
        wt = np.asarray(w, f32).T.reshape(4, 2, 128, 128)  # [e, s, p, m]
        return wt.transpose(2, 0, 1, 3)                    # [p, e, s, m]

    w1pk_h = np.empty((128, 2, 4, 2, 128), f32)
    w1pk_h[:, 0] = w1f8(w_off1)
    w1pk_h[:, 1] = w1f8(w_att1)
    w1pk_h = w1pk_h.reshape(128, 2048).astype(f8)

    w2o_h = np.ascontiguousarray(
        np.concatenate([w_off2[0::2], w_off2[1::2]], 0).T.astype(bf)
    )
    w2a_h = np.ascontiguousarray(np.asarray(w_att2, f32).T.astype(bf))

    bpack_h = np.zeros((128, BP_TOT), bf)
    bpack_h[0:8, BP_ID8:BP_ID8 + 8] = np.eye(8, dtype=f32).astype(bf)
    bpack_h[:, BP_W2O:BP_W2O + 8] = w2o_h
    bpack_h[:, BP_W2A:BP_W2A + 4] = w2a_h

    # host-side keypoint geometry (from keypoint_coords only)
    kp = np.asarray(keypoint_coords, f32)           # [32, J, 2]
    ix = (kp[..., 0] + 1.0) * 31.5                  # [32, J]
    iy = (kp[..., 1] + 1.0) * 31.5
    x0 = np.floor(ix); y0 = np.floor(iy)
    fx = ix - x0; fy = iy - y0
    bx = np.clip(np.round(ix) - 1.0, 0.0, 61.0)
    by = np.clip(np.round(iy) - 1.0, 0.0, 61.0)
    isB = (fy < DELTA) | (fy > 1.0 - DELTA)         # needs 3-row window
    ylo = np.where(isB, by, y0)                     # per-keypoint y base

    PIW = 16 + NBCAP // 16

    in_maps = []
    for m in range(n_cores):
        bs = slice(B * m, B * (m + 1))
        fq = np.asarray(features[bs], f32).transpose(0, 2, 3, 1).reshape(
            B * HW, C)
        feat_h = np.ascontiguousarray(fq).astype(bf)

        # per-b sigma order: B-class keypoints first
        sig = np.empty((B, J), np.int64)
        nBs = np.empty(B, np.int64)
        for b in range(B):
            gb = B * m + b
            order = np.argsort(~isB[gb], kind="stable")
            sig[b] = order
            nBs[b] = int(isB[gb].sum())
            assert nBs[b] <= NBCAP, f"nB={nBs[b]} exceeds NBCAP"

        def sg(arr):  # [32, J] -> [J, B] sigma-gathered for this core
            outv = np.empty((J, B), f32)
            for b in range(B):
                outv[:, b] = arr[B * m + b][sig[b]]
            return outv

        ixs = sg(ix); iys = sg(iy); bxs = sg(bx); ylos = sg(ylo)
        x0s = sg(x0); y0s = sg(y0); fxs = sg(fx); fys = sg(fy)

        bpc = bpack_h.copy()
        # seed idx: flat order (y, b, j_sigma): i = (y0+y)*64 + x0 + b*HW
        sflat = np.empty(2 * J * B, np.int16)
        for y in range(2):
            for b in range(B):
                sflat[y * J * B + b * J:y * J * B + (b + 1) * J] = (
                    (y0s[:, b] + y) * 64.0 + x0s[:, b] + b * HW
                ).astype(np.int16)
        bpc[:, BP_SIDX:BP_SIDX + 64] = wrap(sflat).view(bf)
        # patch idx per b: [row0 all j | row1 all j] then [row2 first NBCAP]
        for b in range(B):
            cell0 = (ylos[:, b] * 64.0 + bxs[:, b] + b * HW).astype(np.int16)
            main = np.concatenate([cell0, cell0 + 64]).astype(np.int16)
            bpc[:, BP_PIDX + b * PIW:BP_PIDX + b * PIW + 16] = (
                wrap(main).view(bf))
            row2 = np.zeros(NBCAP, np.int16)  # idx 0 = harmless valid dummy
            row2[:nBs[b]] = cell0[:nBs[b]] + 128
            bpc[:, BP_PIDX + b * PIW + 16:BP_PIDX + (b + 1) * PIW] = (
                wrap(row2).view(bf))

        # seed weights wsg[x, y, b*J+j_sigma] = wx(x)*wy(y)
        wsg_h = np.empty((2, 2, J * B), f32)
        for x in range(2):
            for y in range(2):
                wx = (1.0 - fxs) if x == 0 else fxs
                wy = (1.0 - fys) if y == 0 else fys
                wsg_h[x, y] = (wx * wy).T.reshape(J * B)

        # permutation matrices: P[k_sigma, orig_j] = 1
        ppack_h = np.zeros((128, 4 * 128), f32)
        for b in range(B):
            ppack_h[np.arange(J), b * 128 + sig[b]] = 1.0

        fpack_h = np.zeros((128, 24), f32)
        fpack_h[:, 0:4] = ixs
        fpack_h[:, 4:8] = iys
        fpack_h[:, 8:12] = bxs
        fpack_h[:, 12:16] = ylos
        fpack_h[:, 16] = np.asarray(b_off1, f32)
        fpack_h[:, 17] = np.asarray(b_att1, f32)
        fpack_h[:, 18:22] = np.arange(4, dtype=f32)[None, :]
        fpack_h[0:8, 22] = np.concatenate(
            [b_off2[0::2], b_off2[1::2]]).astype(f32)
        fpack_h[0:4, 23] = np.asarray(b_att2, f32)

        in_maps.append({
            "feat": feat_h,
            "wsg": np.broadcast_to(
                wsg_h.reshape(-1).astype(bf)[None, :], (128, 2048)
            ).copy(),
            "bpack": bpc, "fpack": fpack_h, "w1pk": w1pk_h,
            "ppack": ppack_h.astype(bf),
        })
    return in_maps


_NC_CACHE = None


def get_nc():
    global _NC_CACHE
    if _NC_CACHE is None:
        _NC_CACHE = build_nc()
    return _NC_CACHE


def kernel(**inputs):
    from concourse.bass_utils import run_bass_kernel_spmd

    n_cores = 8
    nc = get_nc()
    in_maps = prepare_in_maps(**inputs, n_cores=n_cores)
    res = run_bass_kernel_spmd(
        nc, in_maps, core_ids=list(range(n_cores)),
        trace=bool(int(os.environ.get("KERNEL_TRACE", "0") or 0)),
    )
    kernel.last_results = res
    outs = [
        np.asarray(r["out"]).astype(np.float32).reshape(B, J, C)
        for r in res.results
    ]
    return np.concatenate(outs, axis=0)


# revision 50
# speedup vs baseline: 1.0261x; 1.0261x over previous
"""Trainium2 Bass kernel for nn_AdaptiveSampler (sparse grid_sample attention).

Strategy v4.7 (data-parallel over batch, 8 cores x 4 batch items each):
  - Host: features channels-last bf16 in DRAM; all gather indices derive
    from keypoint_coords only and are precomputed on host.
  - Seed path: bf16 channel-major transposed gathers (2-cell rows x 2 y
    rows), DVE bilinear combine in 2x mode, y-add writes the seed as fp8
    in (e', s, i) pair layout, then the C->128 MLP layer runs as fp8
    DoubleRow matmuls (2x column rate).
  - Patch path y-adaptive: keypoints whose frac(iy) is within DELTA of an
    integer boundary ("B class") need a 3-row window; the rest ("A") need
    only 2 rows (saves ~23% of patch bytes). Per batch item the keypoints
    are permuted B-first; one gather per b fetches [row0 x128 | row1 x128 |
    row2 x NBCAP]. The permutation back to original keypoint order is
    folded into the fuse stationaries (permutation matrix instead of
    identity), so it costs nothing.
  - Fuse weights use the exact tent form relu(1 - |p - cell|) (no
    floor/is_equal chains); softmax runs without the max-subtraction
    (logits are tiny). DVE emission order is tuned so weights_pair(0),
    which gates the whole fuse chain, runs before the half-1 combine tail.
  - Fuse: 9 permuted-diag matmuls per batch item on PE (bf16); the 3
    third-row cells contract only over the first NBCAP=64 partitions.
  - Gathers: all 4 seed calls first, then the 4 patch calls — the seed-fed
    MLP/weights pipeline then overlaps the patch drains, and every fuse is
    gated only by its own patch arrival. Gather calls are merged (8 total)
    because each SWDGE call carries ~1-3.5us of serial overhead; transposed
    gathers above 256 idxs hang the hardware, so seeds stay at 4 calls.
"""

import os
import sys
from contextlib import ExitStack

import numpy as np

sys.path.insert(0, "/opt/trn_rl_repo")

import ml_dtypes

import concourse.bass as bass
import concourse.tile as tile
from concourse import bacc, mybir

F32 = mybir.dt.float32
BF16 = mybir.dt.bfloat16
F8 = mybir.dt.float8e4
I16 = mybir.dt.int16

ALU = mybir.AluOpType
ACT = mybir.ActivationFunctionType
AX = mybir.AxisListType
DR = mybir.MatmulPerfMode.DoubleRow

B = 4          # batch items per core
C = 1024       # channels
H = W = 64
HW = H * W     # 4096 cells per batch item
J = 128        # keypoints
NP = 4         # sample points per keypoint
TWO23 = float(2 ** 23)

DELTA = 0.12   # y-window classification margin (max |offset| is ~0.080)
NBCAP = 48     # static cap on 3-row keypoints per batch item (max seen: 39).
               # K=48 row-2 matmuls round up to the same (64, 128) tile size,
               # and rows 0..47 of the third stripe are gather-initialized.

# bpack bf16 column map
BP_ID8 = 0        # eye(8) rows 0:8
BP_W2O = 8        # [8]
BP_W2A = 16       # [4]
BP_SIDX = 20      # seed idx 64 cols
BP_PIDX = 84      # patch idx 4 * (16 + NBCAP//16) cols
BP_TOT = 84 + 4 * (16 + NBCAP // 16)


def build_nc():
    nc = bacc.Bacc()

    feat = nc.declare_dram_parameter("feat", [B * HW, C], BF16, isOutput=False)
    # seed-combine weights wsg[x, y, jb] replicated across partitions
    wsg = nc.declare_dram_parameter("wsg", [128, 2 * 2 * J * B], BF16,
                                    isOutput=False)
    # fp8 W1 pack: [head(2), e(4), s(2), m(128)] per partition row
    w1pk_d = nc.declare_dram_parameter("w1pk", [128, 2048], F8, isOutput=False)
    # per-b permutation matrices (sigma-row -> original keypoint)
    ppack = nc.declare_dram_parameter("ppack", [128, 4 * 128], BF16,
                                      isOutput=False)
    # bf16 pack: [id8 | w2o 8 | w2a 4 | seed idx 64 | patch idx]
    bpack = nc.declare_dram_parameter("bpack", [128, BP_TOT], BF16,
                                      isOutput=False)
    # f32 pack: [ix 4 | iy 4 | bx 4 | ylo 4 | b1o | b1a | posc 4 | b2o | b2a]
    fpack = nc.declare_dram_parameter("fpack", [128, 24], F32, isOutput=False)
    out = nc.declare_dram_parameter("out", [B * J, C], BF16, isOutput=True)

    # Overlapping row views. 3-cell rows (patch): max start idx 16381.
    # 2-cell rows (seed): max start 16382.
    feat_ov3 = bass.AP(feat[:].tensor, 0, [[C, B * HW - 2], [1, 3 * C]])
    feat_ov2 = bass.AP(feat[:].tensor, 0, [[C, B * HW - 1], [1, 2 * C]])

    with ExitStack() as ctx:
        tc = ctx.enter_context(tile.TileContext(nc))
        cons = ctx.enter_context(tc.tile_pool(name="cons", bufs=1))
        gp = ctx.enter_context(tc.tile_pool(name="gpool", bufs=1))
        a = ctx.enter_context(tc.tile_pool(name="work", bufs=1))
        dgp = ctx.enter_context(tc.tile_pool(name="diag", bufs=2))
        ps = ctx.enter_context(tc.tile_pool(name="psT", bufs=2, space="PSUM"))
        pmm = ctx.enter_context(tc.tile_pool(name="psMM", bufs=2, space="PSUM"))
        pfu = ctx.enter_context(tc.tile_pool(name="psFU", bufs=2, space="PSUM"))

        # ---------------- constants ----------------
        # The idx-bearing pack goes first so the gathers unblock ASAP.
        bpk = cons.tile([128, BP_TOT], BF16, tag="bpk")
        nc.scalar.dma_start(out=bpk[:], in_=bpack[:])
        sidx_sb = bpk[:, BP_SIDX:BP_SIDX + 64].bitcast(I16)
        pidx_sb = bpk[:, BP_PIDX:BP_TOT].bitcast(I16)
        PIW = 16 + NBCAP // 16  # idx cols per batch item

        # ---------------- gathers ----------------------------------------
        # Seed chunks (bf16, channel-major, transposed): h -> y = h//2,
        # jb half = h%2, 256 idxs each.
        # Patch (keypoint-major, y-adaptive): one call per b of
        # [row0 x128 | row1 x128 | row2 x NBCAP]; rows 0..NBCAP-1 of the
        # third stripe are always gather-initialized and the row-2 fuse
        # matmuls contract K=NBCAP only.
        G2h = [None] * 4
        Gt = [None] * B

        def seed_gather(h):
            # bf16 so the DVE combine runs in 2x mode; channel = 128*e16 + p
            # with e16 = x*8 + q.
            g2 = gp.tile([128, 16, 256], BF16, tag=f"G2{h}")
            nc.gpsimd.dma_gather(
                g2[:],
                feat_ov2,
                sidx_sb[:, 16 * h:16 * h + 16],
                num_idxs=256,
                num_idxs_reg=r256,
                elem_size=2 * C,
                elem_step=C,
                transpose=True,
            )
            G2h[h] = g2

        def patch_gather(b):
            g = gp.tile([128, 3, 3 * C], BF16, tag=f"G{b}")
            nc.gpsimd.dma_gather(
                g[:],
                feat_ov3,
                pidx_sb[:, b * PIW:(b + 1) * PIW],
                num_idxs=256 + NBCAP,
                num_idxs_reg=r320,
                elem_size=3 * C,
                elem_step=C,
                transpose=False,
            )
            Gt[b] = g

        r256 = nc.gpsimd.to_reg(256)
        r320 = nc.gpsimd.to_reg(256 + NBCAP)

        seed_gather(0)
        seed_gather(2)
        seed_gather(1)
        seed_gather(3)
        patch_gather(0)
        patch_gather(1)
        patch_gather(2)
        patch_gather(3)

        # ---------------- remaining constants (overlap the gathers) -------
        fpk = cons.tile([128, 24], F32, tag="fpk")
        nc.sync.dma_start(out=fpk[:], in_=fpack[:])
        wsg_flat = cons.tile([128, 2 * 2 * J * B], BF16, tag="wsg")
        nc.sync.dma_start(out=wsg_flat[:], in_=wsg[:])
        wsg_sb = wsg_flat[:].rearrange("p (x y i) -> p x y i", x=2, y=2)
        w1pk = cons.tile([128, 2048], F8, tag="w1pk")
        nc.sync.dma_start(out=w1pk[:], in_=w1pk_d[:])
        w1v = w1pk[:].rearrange("p (hd e s m) -> p hd e s m", hd=2, e=4, s=2,
                                m=128)
        ppk = cons.tile([128, 512], BF16, tag="ppk")
        nc.sync.dma_start(out=ppk[:], in_=ppack[:])

        id8_sb = bpk[0:8, 0:8]
        w2o_sb = bpk[:, BP_W2O:BP_W2O + 8]
        w2a_sb = bpk[:, BP_W2A:BP_W2A + 4]
        b1o_sb = fpk[:, 16:17]
        b1a_sb = fpk[:, 17:18]
        posc_sb = fpk[:, 18:22]
        b2o_sb = fpk[0:8, 22:23]
        b2a_sb = fpk[0:4, 23:24]

        ixv = fpk[:, 0:4]    # [J, B] pixel x coords (sigma order)
        iyv = fpk[:, 4:8]
        bxv = fpk[:, 8:12]   # patch x base (f32 integer-valued)
        ylov = fpk[:, 12:16]  # patch y base (y0 for A class, by for B class)

        # ---------------- seed combine (DVE, fp8) ------------------------
        # G2 layout: [p, x(2), e(4), i(256), s(2)], channel = 2*(128e+p)+s
        seedh = []
        for jh in range(2):
            sh = a.tile([128, 4, 256, 2], F8, tag=f"seed{jh}")
            seedh.append(sh)

        comb_t = {}

        def combine_y(jh, y):
            g2 = G2h[2 * y + jh]
            vg = (g2[:].rearrange("p a i -> p (a i)")
                  .rearrange("p (x q i) -> p x q i", x=2, q=8, i=256))
            t = a.tile([128, 8, 256], BF16, tag=f"tcomb{jh}_{y}")
            t2 = a.tile([128, 8, 256], BF16, tag=f"tcomb2{jh}_{y}")
            for x in range(2):
                wb = (wsg_sb[:, x, y, 256 * jh:256 * jh + 256]
                      .unsqueeze(1).to_broadcast((128, 8, 256)))
                nc.vector.tensor_tensor(
                    (t if x == 0 else t2)[:], vg[:, x], wb, ALU.mult)
            nc.vector.tensor_tensor(t[:], t[:], t2[:], ALU.add)
            comb_t[(jh, y)] = t

        def combine_fin(jh):
            # y-add writes the fp8 seed in (e', s, i) pair layout for the
            # DoubleRow MLP: channel = 128*(2e'+s) + p, q = 2e'+s.
            sv = (seedh[jh][:].rearrange("p e i s -> p (e i s)")
                  .rearrange("p (e i s) -> p e s i", e=4, i=256, s=2))
            t0v = comb_t[(jh, 0)][:].rearrange("p (e s) i -> p e s i",
                                               e=4, s=2)
            t1v = comb_t[(jh, 1)][:].rearrange("p (e s) i -> p e s i",
                                               e=4, s=2)
            nc.vector.tensor_tensor(sv, t0v, t1v, ALU.add)

        def combine_half(jh):
            combine_y(jh, 0)
            combine_y(jh, 1)
            combine_fin(jh)

        # DVE emission order matters (in-order engine): combine half-0 fully,
        # then only the y0-part of half-1, so weights_pair(0) — which gates
        # the whole fuse chain — runs as soon as the half-0 MLP lands.
        with nc.allow_low_precision("fp8 grid-sample compute"):
            combine_half(0)
            combine_y(1, 0)

        # ---------------- MLP + w9 weights, per jb-half pipeline ----------
        offT = a.tile([J, B, 8], F32)
        attT = a.tile([J, B, 4], F32)

        def mlp_chain(jh):
            seedv = seedh[jh][:].rearrange("p e i s -> p e s i")

            def head(hd, b1_sb, w2_sb, b2_sb, m2, name):
                hps = pmm.tile([128, 256], F32, tag="mlp")
                for e in range(4):
                    nc.tensor.matmul(
                        hps[:], w1v[:, hd, e], seedv[:, e],
                        start=(e == 0), stop=(e == 3), perf_mode=DR,
                    )
                h_sb = a.tile([128, 256], BF16, tag=f"hsb_{name}{jh}")
                nc.scalar.activation(h_sb[:], hps[:], ACT.Relu, bias=b1_sb)
                ps2 = pmm.tile([m2, 256], F32, tag="mlp")
                nc.tensor.matmul(ps2[:], w2_sb, h_sb[:], start=True, stop=True)
                o2 = a.tile([m2, 256], BF16, tag=f"o2_{name}{jh}")
                nc.scalar.activation(o2[:], ps2[:], ACT.Identity, bias=b2_sb)
                return o2

            off2 = head(0, b1o_sb, w2o_sb, b2o_sb, 8, "off")
            att2 = head(1, b1a_sb, w2a_sb, b2a_sb, 4, "att")
            for bl in range(2):
                b = 2 * jh + bl
                pso = ps.tile([128, 8], BF16, tag="tp")
                nc.tensor.transpose(
                    pso[:, 0:8], off2[:, bl * J:(bl + 1) * J], id8_sb)
                nc.scalar.copy(offT[:, b, :], pso[:, 0:8])
                psa = ps.tile([128, 4], BF16, tag="tp")
                nc.tensor.transpose(
                    psa[:, 0:4], att2[:, bl * J:(bl + 1) * J],
                    id8_sb[0:4, 0:4])
                nc.scalar.copy(attT[:, b, :], psa[:, 0:4])

        def weights_pair(jh):
            """Fuse weights for batch pair (2jh, 2jh+1) -> w9 f32 [J,2,3,3].

            Axis weights use the exact tent form relu(1 - |p - cell|), which
            equals the reference's position-select bilinear weights including
            the out-of-window and clipped-border zero cases.
            """
            bsl = slice(2 * jh, 2 * jh + 2)
            # pxy[j, b, n, ax]: sample positions for both axes
            pxy = a.tile([J, 2, NP, 2], F32, tag=f"pxy{jh}")
            nc.vector.tensor_tensor(
                pxy[:, :, :, 0],
                ixv[:, bsl].unsqueeze(2).to_broadcast((J, 2, NP)),
                offT[:, bsl, 0:NP],
                ALU.add,
            )
            nc.vector.tensor_tensor(
                pxy[:, :, :, 1],
                iyv[:, bsl].unsqueeze(2).to_broadcast((J, 2, NP)),
                offT[:, bsl, NP:2 * NP],
                ALU.add,
            )
            # basepos[j, b, ax, pos] = window base + pos
            posb3 = posc_sb[:, 0:3].unsqueeze(1).to_broadcast((J, 2, 3))
            basepos = a.tile([J, 2, 2, 3], F32, tag=f"bp{jh}")
            nc.vector.tensor_tensor(
                basepos[:, :, 0, :],
                bxv[:, bsl].unsqueeze(2).to_broadcast((J, 2, 3)), posb3,
                ALU.add,
            )
            nc.vector.tensor_tensor(
                basepos[:, :, 1, :],
                ylov[:, bsl].unsqueeze(2).to_broadcast((J, 2, 3)), posb3,
                ALU.add,
            )
            # tent: sel = relu(1 - |pxy - basepos|)  [J, 2, NP, 2, 3]
            sel = a.tile([J, 2, NP, 2, 3], F32, tag=f"sel{jh}")
            nc.vector.tensor_tensor(
                sel[:],
                pxy[:].unsqueeze(4).to_broadcast((J, 2, NP, 2, 3)),
                basepos[:].unsqueeze(2).to_broadcast((J, 2, NP, 2, 3)),
                ALU.subtract,
            )
            neg = a.tile([J, 2, NP, 2, 3], F32, tag=f"neg{jh}")
            nc.vector.tensor_scalar(neg[:], sel[:], -1.0, 0.0, ALU.mult,
                                    ALU.add)
            nc.vector.tensor_tensor(sel[:], sel[:], neg[:], ALU.max)
            nc.vector.tensor_scalar(sel[:], sel[:], -1.0, 1.0, ALU.mult,
                                    ALU.add)
            nc.vector.tensor_scalar_max(sel[:], sel[:], 0.0)
            wxsel = sel[:, :, :, 0, :]
            wysel = sel[:, :, :, 1, :]
            amax = a.tile([J, 2, 1], F32, tag=f"amax{jh}")
            nc.vector.tensor_reduce(amax[:], attT[:, bsl, :], AX.X, ALU.max)
            ae = a.tile([J, 2, NP], F32, tag=f"ae{jh}")
            nc.vector.tensor_tensor(
                ae[:], attT[:, bsl, :], amax[:].to_broadcast((J, 2, NP)),
                ALU.subtract,
            )
            nc.scalar.activation(ae[:], ae[:], ACT.Exp)
            asum = a.tile([J, 2, 1], F32, tag=f"asum{jh}")
            nc.vector.tensor_reduce(asum[:], ae[:], AX.X, ALU.add)
            nc.vector.reciprocal(asum[:], asum[:])
            attw = a.tile([J, 2, NP], F32, tag=f"attw{jh}")
            nc.vector.tensor_tensor(
                attw[:], ae[:], asum[:].to_broadcast((J, 2, NP)), ALU.mult
            )
            ty = a.tile([J, 2, NP, 3], F32, tag=f"ty{jh}")
            nc.vector.tensor_tensor(
                ty[:], wysel,
                attw[:].unsqueeze(3).to_broadcast((J, 2, NP, 3)),
                ALU.mult,
            )
            w9 = a.tile([J, 2, 3, 3], F32, tag=f"w9{jh}")
            tmp9 = a.tile([J, 2, 3, 3], F32, tag=f"tmp9{jh}")
            for n in range(NP):
                dst = (w9 if n == 0 else tmp9)
                nc.vector.tensor_tensor(
                    dst[:],
                    ty[:, :, n, :].unsqueeze(3).to_broadcast((J, 2, 3, 3)),
                    wxsel[:, :, n, :].unsqueeze(2).to_broadcast((J, 2, 3, 3)),
                    ALU.mult,
                )  # wxsel slice dims [J, 2, 3] -> bcast over y
                if n > 0:
                    nc.vector.tensor_tensor(w9[:], w9[:], tmp9[:], ALU.add)
            return w9

        # ---------------- fuse: permuted-diag matmuls ---------------------
        def build_dgs(b, w9, bl):
            p_sb = ppk[:, 128 * b:128 * (b + 1)]
            dgs = []
            for k in range(9):
                dg = dgp.tile([128, 128], BF16, tag=f"dg{k}",
                              padded_shape=[128, 1024])
                nc.scalar.activation(
                    dg[:], p_sb, ACT.Identity,
                    scale=w9[:, bl, k // 3, k % 3:k % 3 + 1],
                )
                dgs.append(dg)
            return dgs

        def fuse_b(b, dgs):
            # acc0's nine cells complete before acc1 starts, so the first
            # half of the output evacuates while acc1 still accumulates.
            acc0 = pfu.tile([128, 512], F32, tag="facc0")
            acc1 = pfu.tile([128, 512], F32, tag="facc1")
            for acc, hh in ((acc0, 0), (acc1, 1)):
                for k in range(9):
                    y, x = k // 3, k % 3
                    kext = 128 if y < 2 else NBCAP
                    nc.tensor.matmul(
                        acc[:], dgs[k][0:kext, :],
                        Gt[b][0:kext, y,
                              x * C + hh * 512:x * C + hh * 512 + 512],
                        start=(k == 0), stop=(k == 8),
                        skip_group_check=True,
                    )
            return acc0, acc1

        def out_b(b, acc0, acc1):
            # acc0 evacuates on the DVE (idle by the time fuses run) in
            # parallel with the Scalar copy of acc1.
            fo = a.tile([128, C], BF16, tag=f"fo{b}")
            nc.vector.tensor_copy(fo[:, 0:512], acc0[:])
            nc.scalar.copy(fo[:, 512:1024], acc1[:])
            nc.sync.dma_start(out=out[b * J:(b + 1) * J, :], in_=fo[:])

        mlp_chain(0)
        w90 = weights_pair(0)
        with nc.allow_low_precision("fp8 grid-sample compute"):
            combine_y(1, 1)
            combine_fin(1)
        mlp_chain(1)
        w91 = weights_pair(1)
        dgs0 = build_dgs(0, w90, 0)
        a0, a1 = fuse_b(0, dgs0)
        dgs1 = build_dgs(1, w90, 1)
        b0, b1 = fuse_b(1, dgs1)
        out_b(0, a0, a1)
        dgs2 = build_dgs(2, w91, 0)
        c0, c1 = fuse_b(2, dgs2)
        out_b(1, b0, b1)
        dgs3 = build_dgs(3, w91, 1)
        d0, d1 = fuse_b(3, dgs3)
        out_b(2, c0, c1)
        out_b(3, d0, d1)

    nc.finalize()
    return nc


def prepare_in_maps(features, keypoint_coords, w_off1, b_off1, w_off2, b_off2,
                    w_att1, b_att1, w_att2, b_att2, n_cores=8):
    bf = ml_dtypes.bfloat16
    f8 = ml_dtypes.float8_e4m3
    f32 = np.float32

    def wrap(flat):  # [N] int16 -> [128, N//16] gpsimd wrapped layout
        n = flat.shape[0]
        return np.tile(flat.reshape(n // 16, 16).T, (8, 1))

    # W1 pack: [p, head, e, s, m] with channel(e, p, s) = 128*(2e+s)+p
    def w1f8(w):  # [128m, C] -> [128p, 4e, 2s, 128m]
        wt = np.asarray(w, f32).T.reshape(4, 2, 128, 128)  # [e, s, p, m]
        return wt.transpose(2, 0, 1, 3)                    # [p, e, s, m]

    w1pk_h = np.empty((128, 2, 4, 2, 128), f32)
    w1pk_h[:, 0] = w1f8(w_off1)
    w1pk_h[:, 1] = w1f8(w_att1)
    w1pk_h = w1pk_h.reshape(128, 2048).astype(f8)

    w2o_h = np.ascontiguousarray(
        np.concatenate([w_off2[0::2], w_off2[1::2]], 0).T.astype(bf)
    )
    w2a_h = np.ascontiguousarray(np.asarray(w_att2, f32).T.astype(bf))

    bpack_h = np.zeros((128, BP_TOT), bf)
    bpack_h[0:8, BP_ID8:BP_ID8 + 8] = np.eye(8, dtype=f32).astype(bf)
    bpack_h[:, BP_W2O:BP_W2O + 8] = w2o_h
    bpack_h[:, BP_W2A:BP_W2A + 4] = w2a_h

    # host-side keypoint geometry (from keypoint_coords only)
    kp = np.asarray(keypoint_coords, f32)           # [32, J, 2]
    ix = (kp[..., 0] + 1.0) * 31.5                  # [32, J]
    iy = (kp[..., 1] + 1.0) * 31.5
    x0 = np.floor(ix); y0 = np.floor(iy)
    fx = ix - x0; fy = iy - y0
    bx = np.clip(np.round(ix) - 1.0, 0.0, 61.0)
    by = np.clip(np.round(iy) - 1.0, 0.0, 61.0)
    isB = (fy < DELTA) | (fy > 1.0 - DELTA)         # needs 3-row window
    ylo = np.where(isB, by, y0)                     # per-keypoint y base

    PIW = 16 + NBCAP // 16

    in_maps = []
    for m in range(n_cores):
        bs = slice(B * m, B * (m + 1))
        fq = np.asarray(features[bs], f32).transpose(0, 2, 3, 1).reshape(
            B * HW, C)
        feat_h = np.ascontiguousarray(fq).astype(bf)

        # per-b sigma order: B-class keypoints first
        sig = np.empty((B, J), np.int64)
        nBs = np.empty(B, np.int64)
        for b in range(B):
            gb = B * m + b
            order = np.argsort(~isB[gb], kind="stable")
            sig[b] = order
            nBs[b] = int(isB[gb].sum())
            assert nBs[b] <= NBCAP, f"nB={nBs[b]} exceeds NBCAP"

        def sg(arr):  # [32, J] -> [J, B] sigma-gathered for this core
            outv = np.empty((J, B), f32)
            for b in range(B):
                outv[:, b] = arr[B * m + b][sig[b]]
            return outv

        ixs = sg(ix); iys = sg(iy); bxs = sg(bx); ylos = sg(ylo)
        x0s = sg(x0); y0s = sg(y0); fxs = sg(fx); fys = sg(fy)

        bpc = bpack_h.copy()
        # seed idx: flat order (y, b, j_sigma): i = (y0+y)*64 + x0 + b*HW
        sflat = np.empty(2 * J * B, np.int16)
        for y in range(2):
            for b in range(B):
                sflat[y * J * B + b * J:y * J * B + (b + 1) * J] = (
                    (y0s[:, b] + y) * 64.0 + x0s[:, b] + b * HW
                ).astype(np.int16)
        bpc[:, BP_SIDX:BP_SIDX + 64] = wrap(sflat).view(bf)
        # patch idx per b: [row0 all j | row1 all j] then [row2 first NBCAP]
        for b in range(B):
            cell0 = (ylos[:, b] * 64.0 + bxs[:, b] + b * HW).astype(np.int16)
            main = np.concatenate([cell0, cell0 + 64]).astype(np.int16)
            bpc[:, BP_PIDX + b * PIW:BP_PIDX + b * PIW + 16] = (
                wrap(main).view(bf))
            row2 = np.zeros(NBCAP, np.int16)  # idx 0 = harmless valid dummy
            row2[:nBs[b]] = cell0[:nBs[b]] + 128
            bpc[:, BP_PIDX + b * PIW + 16:BP_PIDX + (b + 1) * PIW] = (
                wrap(row2).view(bf))

        # seed weights wsg[x, y, b*J+j_sigma] = wx(x)*wy(y)
        wsg_h = np.empty((2, 2, J * B), f32)
        for x in range(2):
            for y in range(2):
                wx = (1.0 - fxs) if x == 0 else fxs
                wy = (1.0 - fys) if y == 0 else fys
                wsg_h[x, y] = (wx * wy).T.reshape(J * B)

        # permutation matrices: P[k_sigma, orig_j] = 1
        ppack_h = np.zeros((128, 4 * 128), f32)
        for b in range(B):
            ppack_h[np.arange(J), b * 128 + sig[b]] = 1.0

        fpack_h = np.zeros((128, 24), f32)
        fpack_h[:, 0:4] = ixs
        fpack_h[:, 4:8] = iys
        fpack_h[:, 8:12] = bxs
        fpack_h[:, 12:16] = ylos
        fpack_h[:, 16] = np.asarray(b_off1, f32)
        fpack_h[:, 17] = np.asarray(b_att1, f32)
        fpack_h[:, 18:22] = np.arange(4, dtype=f32)[None, :]
        fpack_h[0:8, 22] = np.concatenate(
            [b_off2[0::2], b_off2[1::2]]).astype(f32)
        fpack_h[0:4, 23] = np.asarray(b_att2, f32)

        in_maps.append({
            "feat": feat_h,
            "wsg": np.broadcast_to(
                wsg_h.reshape(-1).astype(bf)[None, :], (128, 2048)
            ).copy(),
            "bpack": bpc, "fpack": fpack_h, "w1pk": w1pk_h,
            "ppack": ppack_h.astype(bf),
        })
    return in_maps


_NC_CACHE = None


def get_nc():
    global _NC_CACHE
    if _NC_CACHE is None:
        _NC_CACHE = build_nc()
    return _NC_CACHE


def kernel(**inputs):
    from concourse.bass_utils import run_bass_kernel_spmd

    n_cores = 8
    nc = get_nc()
    in_maps = prepare_in_maps(**inputs, n_cores=n_cores)
    res = run_bass_kernel_spmd(
        nc, in_maps, core_ids=list(range(n_cores)),
        trace=bool(int(os.environ.get("KERNEL_TRACE", "0") or 0)),
    )
    kernel.last_results = res
    outs = [
        np.asarray(r["out"]).astype(np.float32).reshape(B, J, C)
        for r in res.results
    ]
    return np.concatenate(outs, axis=0)
